# revision 2
# baseline (speedup 1.0000x reference)
"""Trainium2 Bass kernel: fused gather + segment-mean (GraphSAGE-style mean
aggregator).

out[t] = mean_j features[neighbor_idx[t, j]]   t in [0, 100000), j in [0, 32)

Strategy: shard the 100k target rows across 8 NeuronCores (12.5k each);
replicate the 1M x 128 feature table in each core's DRAM. Per 128-target
block, one indirect DMA gathers the 128*32 neighbor rows (512B each) into a
[128, 32*128] SBUF tile (partition = target), then a single strided
vector-engine reduce sums over the 32 samples. Features are pre-scaled by
1/32 on the host so the reduce directly yields the mean.
"""
import numpy as np

import concourse.bacc as bacc
import concourse.tile as tile
from concourse import bass, mybir
from concourse import bass_utils

P = 128           # partitions / targets per block
D = 128           # feature dim
S = 32            # samples per target
NF = 1_000_000    # feature-table rows
N_TARGETS = 100_000
N_CORES = 8
T_CORE = N_TARGETS // N_CORES          # 12500 targets per core
NITER = (T_CORE + P - 1) // P          # 98 blocks
T_PAD = NITER * P                      # 12544 (44 pad targets)

_CACHE: dict = {}


def _build():
    nc = bacc.Bacc("TRN2", target_bir_lowering=False, debug=False)
    feat = nc.dram_tensor("features", [NF, D], mybir.dt.float32, kind="ExternalInput")
    # idx swizzled on host: idx_dev[p, i*S + j] = neighbor_idx[i*P + p, j]
    idx = nc.dram_tensor("idx", [P, NITER * S], mybir.dt.int32, kind="ExternalInput")
    out = nc.dram_tensor("out", [T_PAD, D], mybir.dt.float32, kind="ExternalOutput")

    with tile.TileContext(nc) as tc:
        with (
            tc.tile_pool(name="io", bufs=1) as iopool,
            tc.tile_pool(name="work", bufs=4) as pool,
        ):
            idx_sb = iopool.tile([P, NITER * S], mybir.dt.int32)
            nc.sync.dma_start(out=idx_sb[:], in_=idx.ap()[:, :])
            for i in range(NITER):
                g = pool.tile([P, S * D], mybir.dt.float32, tag="g")
                for j in range(S):
                    nc.gpsimd.indirect_dma_start(
                        out=g[:, j * D:(j + 1) * D],
                        out_offset=None,
                        in_=feat.ap()[:, :],
                        in_offset=bass.IndirectOffsetOnAxis(
                            ap=idx_sb[:, i * S + j:i * S + j + 1], axis=0
                        ),
                    )
                r = pool.tile([P, D], mybir.dt.float32, tag="r")
                gv = g[:].rearrange("p (j d) -> p d j", j=S, d=D)
                nc.vector.tensor_reduce(
                    out=r[:], in_=gv, axis=mybir.AxisListType.X, op=mybir.AluOpType.add
                )
                nc.sync.dma_start(out=out.ap()[i * P:(i + 1) * P, :], in_=r[:])
    nc.compile()
    return nc


def kernel(features, neighbor_idx):
    features = np.asarray(features, dtype=np.float32)
    neighbor_idx = np.asarray(neighbor_idx).astype(np.int32)
    assert features.shape == (NF, D)
    assert neighbor_idx.shape == (N_TARGETS, S)

    featp = features * np.float32(1.0 / S)

    if "nc" not in _CACHE:
        _CACHE["nc"] = _build()
    nc = _CACHE["nc"]

    in_maps = []
    for c in range(N_CORES):
        ic = neighbor_idx[c * T_CORE:(c + 1) * T_CORE]
        if T_PAD != T_CORE:
            ic = np.concatenate([ic, np.zeros((T_PAD - T_CORE, S), np.int32)], axis=0)
        swz = ic.reshape(NITER, P, S).transpose(1, 0, 2).reshape(P, NITER * S)
        in_maps.append({"features": featp, "idx": np.ascontiguousarray(swz)})

    res = bass_utils.run_bass_kernel_spmd(nc, in_maps, core_ids=list(range(N_CORES)))
    out = np.concatenate([r["out"][:T_CORE] for r in res.results], axis=0)
    return out
